# revision 4
# baseline (speedup 1.0000x reference)
"""Trainium2 Bass kernel for Dempster-Shafer combination of two Dirichlet
parameter maps.

The reference computes, per pixel (N = flattened pixels, C = 21 classes):
    S1 = sum_c alpha1,  S2 = sum_c alpha2
    b1 = (alpha1-1)/S1, b2 = (alpha2-1)/S2, u1 = C/S1, u2 = C/S2
    K  = sum(b1)*sum(b2) - sum(b1*b2), denom = 1-K
    b_a = (b1*b2 + b1*u2 + b2*u1)/denom
    u_a = u1*u2/denom,  S_a = C/u_a
    out = b_a*S_a + 1

The `denom` cancels between b_a and S_a, and S1*S2 cancels against u1*u2,
leaving the exact elementwise identity (with e1 = alpha1-1, e2 = alpha2-1):
    out = e1*e2/C + e1 + e2 + 1
        = (alpha1 - 1) * (alpha2 + C - 1)/C + alpha2
so no per-pixel reductions are needed at all. This factored form is three
on-chip ops per element:
    t   = (alpha2 + C-1)/C      (ScalarE activation: Copy(a2*1/C + (C-1)/C))
    v   = (alpha1 - 1) * t      (VectorE scalar_tensor_tensor)
    out = v + alpha2            (VectorE tensor_tensor add)

Sharding: pure data parallel over the batch dim (8 batches -> 8 cores).
Each core streams its 21*512*512-element f32 shard through SBUF in
[128 x F] tiles.
"""

from contextlib import ExitStack

import numpy as np
import sys

if "/opt/trn_rl_repo" not in sys.path:
    sys.path.insert(0, "/opt/trn_rl_repo")

N_CORES = 8
N_CLASSES = 21
BS, H, W = 8, 512, 512
SHARD_ELEMS = N_CLASSES * H * W  # 5_505_024 = 128 * 43008
P = 128
F = 3072  # free-dim tile size: 128*3072*4B = 1.5 MiB per DMA
NT = SHARD_ELEMS // (P * F)  # 14

_NC_CACHE = {}


def _build_nc(loop_iters: int = 1):
    import concourse.tile as tile
    from concourse import bacc, mybir

    nc = bacc.Bacc(
        "TRN2",
        target_bir_lowering=False,
        debug=False,
        enable_asserts=False,
        num_devices=N_CORES,
    )
    a1 = nc.dram_tensor(
        "alpha1", [SHARD_ELEMS], mybir.dt.float32, kind="ExternalInput"
    ).ap()
    a2 = nc.dram_tensor(
        "alpha2", [SHARD_ELEMS], mybir.dt.float32, kind="ExternalInput"
    ).ap()
    out = nc.dram_tensor(
        "out", [SHARD_ELEMS], mybir.dt.float32, kind="ExternalOutput"
    ).ap()

    a1_t = a1.rearrange("(n p f) -> n p f", p=P, f=F)
    a2_t = a2.rearrange("(n p f) -> n p f", p=P, f=F)
    out_t = out.rearrange("(n p f) -> n p f", p=P, f=F)

    C = float(N_CLASSES)

    def emit_body(tc, pa1, pa2, pt, pv, po):
        for i in range(NT):
            t1 = pa1.tile([P, F], mybir.dt.float32)
            nc.sync.dma_start(t1[:], a1_t[i, :, :])
            t2 = pa2.tile([P, F], mybir.dt.float32)
            nc.sync.dma_start(t2[:], a2_t[i, :, :])

            # t = (a2 + (C-1)) / C on ScalarE
            tt = pt.tile([P, F], mybir.dt.float32)
            nc.scalar.activation(
                tt[:],
                t2[:],
                mybir.ActivationFunctionType.Copy,
                bias=(C - 1.0) / C,
                scale=1.0 / C,
            )
            # v = (a1 - 1) * t on VectorE
            tv = pv.tile([P, F], mybir.dt.float32)
            nc.vector.scalar_tensor_tensor(
                tv[:],
                t1[:],
                1.0,
                tt[:],
                mybir.AluOpType.subtract,
                mybir.AluOpType.mult,
            )
            # out = v + a2 on VectorE
            to = po.tile([P, F], mybir.dt.float32)
            nc.vector.tensor_tensor(
                to[:], tv[:], t2[:], mybir.AluOpType.add
            )
            nc.sync.dma_start(out_t[i, :, :], to[:])

    with ExitStack() as ctx:
        tc = ctx.enter_context(tile.TileContext(nc))
        pa1 = ctx.enter_context(tc.tile_pool(name="pa1", bufs=3))
        pa2 = ctx.enter_context(tc.tile_pool(name="pa2", bufs=3))
        pt = ctx.enter_context(tc.tile_pool(name="pt", bufs=2))
        pv = ctx.enter_context(tc.tile_pool(name="pv", bufs=2))
        po = ctx.enter_context(tc.tile_pool(name="po", bufs=3))
        if loop_iters == 1:
            emit_body(tc, pa1, pa2, pt, pv, po)
        else:
            with tc.For_i(0, loop_iters, 1):
                emit_body(tc, pa1, pa2, pt, pv, po)

    nc.compile()
    return nc


def _get_nc(loop_iters: int = 1):
    if loop_iters not in _NC_CACHE:
        _NC_CACHE[loop_iters] = _build_nc(loop_iters)
    return _NC_CACHE[loop_iters]


def run(inputs: dict, trace: bool = False, loop_iters: int = 1):
    """Run the SPMD kernel on 8 cores. Returns (full_output, BassKernelResults)."""
    from concourse import bass_utils

    nc = _get_nc(loop_iters)
    alpha1 = np.ascontiguousarray(np.asarray(inputs["alpha1"], dtype=np.float32))
    alpha2 = np.ascontiguousarray(np.asarray(inputs["alpha2"], dtype=np.float32))
    assert alpha1.shape == (BS, N_CLASSES, H, W), alpha1.shape
    in_maps = [
        {
            "alpha1": alpha1[c].reshape(SHARD_ELEMS),
            "alpha2": alpha2[c].reshape(SHARD_ELEMS),
        }
        for c in range(N_CORES)
    ]
    res = bass_utils.run_bass_kernel_spmd(
        nc, in_maps, core_ids=list(range(N_CORES)), trace=trace
    )
    out = np.stack(
        [res.results[c]["out"].reshape(N_CLASSES, H, W) for c in range(N_CORES)]
    )
    return out, res


def kernel(alpha1: np.ndarray, alpha2: np.ndarray) -> np.ndarray:
    out, _ = run({"alpha1": alpha1, "alpha2": alpha2})
    return out


# revision 6
# speedup vs baseline: 7.1069x; 7.1069x over previous
"""Trainium2 Bass kernel for Dempster-Shafer combination of two Dirichlet
parameter maps.

The reference computes, per pixel (N = flattened pixels, C = 21 classes):
    S1 = sum_c alpha1,  S2 = sum_c alpha2
    b1 = (alpha1-1)/S1, b2 = (alpha2-1)/S2, u1 = C/S1, u2 = C/S2
    K  = sum(b1)*sum(b2) - sum(b1*b2), denom = 1-K
    b_a = (b1*b2 + b1*u2 + b2*u1)/denom
    u_a = u1*u2/denom,  S_a = C/u_a
    out = b_a*S_a + 1

The `denom` cancels between b_a and S_a, and S1*S2 cancels against u1*u2,
leaving the exact elementwise identity (with e1 = alpha1-1, e2 = alpha2-1):
    out = e1*e2/C + e1 + e2 + 1
        = (alpha2 + C-1) * ((alpha1-1)/C) + alpha2
so no per-pixel reductions are needed at all. Three on-chip ops per element:
    u   = (alpha1 - 1)/C        (ScalarE activation, in-place on the a1 tile)
    v   = (alpha2 + C-1) * u    (VectorE scalar_tensor_tensor)
    out = v + alpha2            (VectorE tensor_tensor add, in-place on v)

Sharding: pure data parallel over the batch dim (8 batches -> 8 cores).
Each core streams its 21*512*512-element f32 shard through SBUF in
[128 x 7168] tiles (3.5 MiB DMAs, 6 tiles/pass, 3 pools x 2 bufs).
The kernel is HBM-bound: ~66 MB/core at the ~358 GB/s per-core limit;
measured ~260-290 GB/s/core with all 8 cores streaming concurrently.
"""

from contextlib import ExitStack

import numpy as np
import sys

if "/opt/trn_rl_repo" not in sys.path:
    sys.path.insert(0, "/opt/trn_rl_repo")

N_CORES = 8
N_CLASSES = 21
BS, H, W = 8, 512, 512
SHARD_ELEMS = N_CLASSES * H * W  # 5_505_024 = 128 * 43008
P = 128
F = 7168  # free-dim tile size: 128*7168*4B = 3.5 MiB per DMA
NT = SHARD_ELEMS // (P * F)  # 6

_NC_CACHE = {}


def _build_nc(loop_iters: int = 1, internal_io: bool = False):
    import concourse.tile as tile
    from concourse import bacc, mybir

    nc = bacc.Bacc(
        "TRN2",
        target_bir_lowering=False,
        debug=False,
        enable_asserts=False,
        num_devices=N_CORES,
    )
    if internal_io:
        seed = nc.dram_tensor(
            "seed", [P, 4], mybir.dt.float32, kind="ExternalInput"
        ).ap()
        probe = nc.dram_tensor(
            "probe", [P, 4], mybir.dt.float32, kind="ExternalOutput"
        ).ap()
        a1 = nc.dram_tensor(
            "A1", [SHARD_ELEMS], mybir.dt.float32, kind="Internal"
        ).ap()
        a2 = nc.dram_tensor(
            "A2", [SHARD_ELEMS], mybir.dt.float32, kind="Internal"
        ).ap()
        out = nc.dram_tensor(
            "OUT", [SHARD_ELEMS], mybir.dt.float32, kind="Internal"
        ).ap()
    else:
        a1 = nc.dram_tensor(
            "alpha1", [SHARD_ELEMS], mybir.dt.float32, kind="ExternalInput"
        ).ap()
        a2 = nc.dram_tensor(
            "alpha2", [SHARD_ELEMS], mybir.dt.float32, kind="ExternalInput"
        ).ap()
        out = nc.dram_tensor(
            "out", [SHARD_ELEMS], mybir.dt.float32, kind="ExternalOutput"
        ).ap()

    a1_t = a1.rearrange("(n p f) -> n p f", p=P, f=F)
    a2_t = a2.rearrange("(n p f) -> n p f", p=P, f=F)
    out_t = out.rearrange("(n p f) -> n p f", p=P, f=F)

    C = float(N_CLASSES)
    with ExitStack() as ctx:
        tc = ctx.enter_context(tile.TileContext(nc))
        pa1 = ctx.enter_context(tc.tile_pool(name="pa1", bufs=2))
        pa2 = ctx.enter_context(tc.tile_pool(name="pa2", bufs=2))
        pv = ctx.enter_context(tc.tile_pool(name="pv", bufs=2))

        def body():
            for i in range(NT):
                t1 = pa1.tile([P, F], mybir.dt.float32)
                nc.sync.dma_start(t1[:], a1_t[i, :, :])
                t2 = pa2.tile([P, F], mybir.dt.float32)
                nc.sync.dma_start(t2[:], a2_t[i, :, :])
                # u = (a1 - 1)/C, in place on the a1 tile
                nc.scalar.activation(
                    t1[:],
                    t1[:],
                    mybir.ActivationFunctionType.Copy,
                    bias=-1.0 / C,
                    scale=1.0 / C,
                )
                # v = (a2 + (C-1)) * u
                tv = pv.tile([P, F], mybir.dt.float32)
                nc.vector.scalar_tensor_tensor(
                    tv[:],
                    t2[:],
                    C - 1.0,
                    t1[:],
                    mybir.AluOpType.add,
                    mybir.AluOpType.mult,
                )
                # out = v + a2, in place on v
                nc.vector.tensor_tensor(
                    tv[:], tv[:], t2[:], mybir.AluOpType.add
                )
                nc.sync.dma_start(out_t[i, :, :], tv[:])

        if internal_io:
            # init the internal streams once so compute engines see sane f32
            psmall = ctx.enter_context(tc.tile_pool(name="psmall", bufs=1))
            ztile = psmall.tile([P, F], mybir.dt.float32)
            nc.vector.memset(ztile[:], 1.5)
            for i in range(NT):
                nc.sync.dma_start(a1_t[i, :, :], ztile[:])
                nc.sync.dma_start(a2_t[i, :, :], ztile[:])

        if loop_iters == 1:
            body()
        else:
            with tc.For_i(0, loop_iters, 1):
                body()

        if internal_io:
            ptile = psmall.tile([P, 4], mybir.dt.float32)
            nc.sync.dma_start(ptile[:], seed[:, :])
            nc.sync.dma_start(ptile[:], out_t[0, :, 0:4])
            nc.sync.dma_start(probe[:, :], ptile[:])

    nc.compile()
    return nc


def _get_nc(loop_iters: int = 1, internal_io: bool = False):
    key = (loop_iters, internal_io)
    if key not in _NC_CACHE:
        _NC_CACHE[key] = _build_nc(loop_iters, internal_io)
    return _NC_CACHE[key]


def run(inputs: dict, loop_iters: int = 1, n_cores: int = N_CORES):
    """Run the SPMD kernel on 8 cores. Returns (full_output, BassKernelResults)."""
    from concourse import bass_utils

    nc = _get_nc(loop_iters)
    alpha1 = np.ascontiguousarray(np.asarray(inputs["alpha1"], dtype=np.float32))
    alpha2 = np.ascontiguousarray(np.asarray(inputs["alpha2"], dtype=np.float32))
    assert alpha1.shape == (BS, N_CLASSES, H, W), alpha1.shape
    in_maps = [
        {
            "alpha1": alpha1[c].reshape(SHARD_ELEMS),
            "alpha2": alpha2[c].reshape(SHARD_ELEMS),
        }
        for c in range(n_cores)
    ]
    res = bass_utils.run_bass_kernel_spmd(
        nc, in_maps, core_ids=list(range(n_cores))
    )
    out = np.stack(
        [res.results[c]["out"].reshape(N_CLASSES, H, W) for c in range(n_cores)]
    )
    return out, res


def bench_hw_time(kbig: int = 1501, reps: int = 6, offset_s: float = 0.21) -> float:
    """Estimate the per-pass HW time (ns) of the streaming body.

    Uses a tiny-IO twin of the kernel (same instruction stream over internal
    DRAM tensors) with the body wrapped in a K-iteration hardware loop, so
    tunnel-transfer noise does not pollute the wall clock. offset_s is the
    fixed per-call RPC overhead measured for K=1 builds (~0.21 s).
    """
    import time

    from concourse import bass_utils

    nc = _get_nc(kbig, internal_io=True)
    in_map = {"seed": np.zeros((P, 4), np.float32)}
    ws = []
    for r in range(reps + 1):
        t0 = time.time()
        bass_utils.run_bass_kernel_spmd(
            nc, [in_map] * N_CORES, core_ids=list(range(N_CORES))
        )
        w = time.time() - t0
        if r > 0:
            ws.append(w)
    return (min(ws) - offset_s) / (kbig - 1) * 1e9


def kernel(alpha1: np.ndarray, alpha2: np.ndarray) -> np.ndarray:
    out, _ = run({"alpha1": alpha1, "alpha2": alpha2})
    return out


# revision 9
# speedup vs baseline: 7.3639x; 1.0362x over previous
"""Trainium2 Bass kernel for Dempster-Shafer combination of two Dirichlet
parameter maps.

The reference computes, per pixel (N = flattened pixels, C = 21 classes):
    S1 = sum_c alpha1,  S2 = sum_c alpha2
    b1 = (alpha1-1)/S1, b2 = (alpha2-1)/S2, u1 = C/S1, u2 = C/S2
    K  = sum(b1)*sum(b2) - sum(b1*b2), denom = 1-K
    b_a = (b1*b2 + b1*u2 + b2*u1)/denom
    u_a = u1*u2/denom,  S_a = C/u_a
    out = b_a*S_a + 1

The `denom` cancels between b_a and S_a, and S1*S2 cancels against u1*u2,
leaving the exact elementwise identity (with e1 = alpha1-1, e2 = alpha2-1):
    out = e1*e2/C + e1 + e2 + 1
        = (alpha2 + C-1) * ((alpha1-1)/C) + alpha2
so no per-pixel reductions are needed at all. Three on-chip ops per element:
    u   = (alpha1 - 1)/C        (ScalarE activation, in-place on the a1 tile)
    v   = (alpha2 + C-1) * u    (VectorE scalar_tensor_tensor)
    out = v + alpha2            (VectorE tensor_tensor add, in-place on v)

Sharding: pure data parallel over the batch dim (8 batches -> 8 cores).
Each core streams its 21*512*512-element f32 shard through SBUF in
[128 x 7168] tiles (3.5 MiB DMAs, 6 tiles/pass, 3 pools x 2 bufs).
The kernel is HBM-bound: ~66 MB/core at the ~358 GB/s per-core limit;
measured ~260-290 GB/s/core with all 8 cores streaming concurrently.
"""

from contextlib import ExitStack

import numpy as np
import sys

if "/opt/trn_rl_repo" not in sys.path:
    sys.path.insert(0, "/opt/trn_rl_repo")

N_CORES = 8
N_CLASSES = 21
BS, H, W = 8, 512, 512
SHARD_ELEMS = N_CLASSES * H * W  # 5_505_024 = 128 * 43008
P = 128
F = 7168  # free-dim tile size: 128*7168*4B = 3.5 MiB per DMA
NT = SHARD_ELEMS // (P * F)  # 6

_NC_CACHE = {}


def _build_nc(loop_iters: int = 1, internal_io: bool = False):
    import concourse.tile as tile
    from concourse import bacc, mybir

    nc = bacc.Bacc(
        "TRN2",
        target_bir_lowering=False,
        debug=False,
        enable_asserts=False,
        num_devices=N_CORES,
    )
    if internal_io:
        seed = nc.dram_tensor(
            "seed", [P, 4], mybir.dt.float32, kind="ExternalInput"
        ).ap()
        probe = nc.dram_tensor(
            "probe", [P, 4], mybir.dt.float32, kind="ExternalOutput"
        ).ap()
        a1 = nc.dram_tensor(
            "A1", [SHARD_ELEMS], mybir.dt.float32, kind="Internal"
        ).ap()
        a2 = nc.dram_tensor(
            "A2", [SHARD_ELEMS], mybir.dt.float32, kind="Internal"
        ).ap()
        out = nc.dram_tensor(
            "OUT", [SHARD_ELEMS], mybir.dt.float32, kind="Internal"
        ).ap()
    else:
        a1 = nc.dram_tensor(
            "alpha1", [SHARD_ELEMS], mybir.dt.float32, kind="ExternalInput"
        ).ap()
        a2 = nc.dram_tensor(
            "alpha2", [SHARD_ELEMS], mybir.dt.float32, kind="ExternalInput"
        ).ap()
        out = nc.dram_tensor(
            "out", [SHARD_ELEMS], mybir.dt.float32, kind="ExternalOutput"
        ).ap()

    a1_t = a1.rearrange("(n p f) -> n p f", p=P, f=F)
    a2_t = a2.rearrange("(n p f) -> n p f", p=P, f=F)
    out_t = out.rearrange("(n p f) -> n p f", p=P, f=F)

    C = float(N_CLASSES)
    with ExitStack() as ctx:
        tc = ctx.enter_context(tile.TileContext(nc))
        pa1 = ctx.enter_context(tc.tile_pool(name="pa1", bufs=2))
        pa2 = ctx.enter_context(tc.tile_pool(name="pa2", bufs=2))
        pv = ctx.enter_context(tc.tile_pool(name="pv", bufs=2))

        def body():
            for i in range(NT):
                t1 = pa1.tile([P, F], mybir.dt.float32)
                nc.sync.dma_start(t1[:], a1_t[i, :, :])
                t2 = pa2.tile([P, F], mybir.dt.float32)
                nc.sync.dma_start(t2[:], a2_t[i, :, :])
                # u = (a1 - 1)/C, in place on the a1 tile. On VectorE
                # (tensor_scalar, 2x f32 mode) rather than ScalarE: keeping
                # the chain on one engine avoids the cross-engine handoff;
                # DVE (~2.5 cyc/elem) still hides under the DMA stream.
                nc.vector.tensor_scalar(
                    t1[:],
                    t1[:],
                    1.0,
                    1.0 / C,
                    mybir.AluOpType.subtract,
                    mybir.AluOpType.mult,
                )
                # v = (a2 + (C-1)) * u
                tv = pv.tile([P, F], mybir.dt.float32)
                nc.vector.scalar_tensor_tensor(
                    tv[:],
                    t2[:],
                    C - 1.0,
                    t1[:],
                    mybir.AluOpType.add,
                    mybir.AluOpType.mult,
                )
                # out = v + a2, in place on v
                nc.vector.tensor_tensor(
                    tv[:], tv[:], t2[:], mybir.AluOpType.add
                )
                nc.sync.dma_start(out_t[i, :, :], tv[:])

        if internal_io:
            # init the internal streams once so compute engines see sane f32
            psmall = ctx.enter_context(tc.tile_pool(name="psmall", bufs=1))
            ztile = psmall.tile([P, F], mybir.dt.float32)
            nc.vector.memset(ztile[:], 1.5)
            for i in range(NT):
                nc.sync.dma_start(a1_t[i, :, :], ztile[:])
                nc.sync.dma_start(a2_t[i, :, :], ztile[:])

        if loop_iters == 1:
            body()
        else:
            with tc.For_i(0, loop_iters, 1):
                body()

        if internal_io:
            ptile = psmall.tile([P, 4], mybir.dt.float32)
            nc.sync.dma_start(ptile[:], seed[:, :])
            nc.sync.dma_start(ptile[:], out_t[0, :, 0:4])
            nc.sync.dma_start(probe[:, :], ptile[:])

    nc.compile()
    return nc


def _get_nc(loop_iters: int = 1, internal_io: bool = False):
    key = (loop_iters, internal_io)
    if key not in _NC_CACHE:
        _NC_CACHE[key] = _build_nc(loop_iters, internal_io)
    return _NC_CACHE[key]


def run(inputs: dict, loop_iters: int = 1, n_cores: int = N_CORES):
    """Run the SPMD kernel on 8 cores. Returns (full_output, BassKernelResults)."""
    from concourse import bass_utils

    nc = _get_nc(loop_iters)
    alpha1 = np.ascontiguousarray(np.asarray(inputs["alpha1"], dtype=np.float32))
    alpha2 = np.ascontiguousarray(np.asarray(inputs["alpha2"], dtype=np.float32))
    assert alpha1.shape == (BS, N_CLASSES, H, W), alpha1.shape
    in_maps = [
        {
            "alpha1": alpha1[c].reshape(SHARD_ELEMS),
            "alpha2": alpha2[c].reshape(SHARD_ELEMS),
        }
        for c in range(n_cores)
    ]
    res = bass_utils.run_bass_kernel_spmd(
        nc, in_maps, core_ids=list(range(n_cores))
    )
    out = np.stack(
        [res.results[c]["out"].reshape(N_CLASSES, H, W) for c in range(n_cores)]
    )
    return out, res


def bench_hw_time(kbig: int = 1501, reps: int = 6, offset_s: float = 0.21) -> float:
    """Estimate the per-pass HW time (ns) of the streaming body.

    Uses a tiny-IO twin of the kernel (same instruction stream over internal
    DRAM tensors) with the body wrapped in a K-iteration hardware loop, so
    tunnel-transfer noise does not pollute the wall clock. offset_s is the
    fixed per-call RPC overhead measured for K=1 builds (~0.21 s).
    """
    import time

    from concourse import bass_utils

    nc = _get_nc(kbig, internal_io=True)
    in_map = {"seed": np.zeros((P, 4), np.float32)}
    ws = []
    for r in range(reps + 1):
        t0 = time.time()
        bass_utils.run_bass_kernel_spmd(
            nc, [in_map] * N_CORES, core_ids=list(range(N_CORES))
        )
        w = time.time() - t0
        if r > 0:
            ws.append(w)
    return (min(ws) - offset_s) / (kbig - 1) * 1e9


def kernel(alpha1: np.ndarray, alpha2: np.ndarray) -> np.ndarray:
    out, _ = run({"alpha1": alpha1, "alpha2": alpha2})
    return out
